# revision 8
# baseline (speedup 1.0000x reference)
import sys

if "/opt/trn_rl_repo" not in sys.path:
    sys.path.insert(0, "/opt/trn_rl_repo")

from contextlib import ExitStack

import numpy as np

import concourse.bacc as bacc
import concourse.tile as tile
from concourse import mybir
from concourse.bass_utils import run_bass_kernel_spmd

F32 = mybir.dt.float32
BF16 = mybir.dt.bfloat16
AF = mybir.ActivationFunctionType
ALU = mybir.AluOpType

N_CORES = 8
B, S, D = 64, 512, 1024
BC = B // N_CORES
KC = D // 128
TC = S // 128
JT = D // 512
SC = S // 128
LN_EPS = 1e-6


def build_nc(bc: int = BC):
    nc = bacc.Bacc("TRN2", target_bir_lowering=False, debug=False,
                   num_devices=N_CORES)
    x = nc.dram_tensor("x", [bc, S, D], F32, kind="ExternalInput").ap()
    W0 = nc.dram_tensor("W0", [D, D], F32, kind="ExternalInput").ap()
    b0 = nc.dram_tensor("b0", [D], F32, kind="ExternalInput").ap()
    W1 = nc.dram_tensor("W1", [D, D], F32, kind="ExternalInput").ap()
    b1 = nc.dram_tensor("b1", [D], F32, kind="ExternalInput").ap()
    Wa = nc.dram_tensor("Wa", [D, 1], F32, kind="ExternalInput").ap()
    ba = nc.dram_tensor("ba", [1], F32, kind="ExternalInput").ap()
    We = nc.dram_tensor("We", [S, S], F32, kind="ExternalInput").ap()
    be = nc.dram_tensor("be", [S], F32, kind="ExternalInput").ap()
    g_out = nc.dram_tensor("greaters", [bc, S], F32, kind="ExternalOutput").ap()
    c_out = nc.dram_tensor("candidate", [bc, S, D], F32,
                           kind="ExternalOutput").ap()

    with tile.TileContext(nc) as tc, ExitStack() as ctx:
        const = ctx.enter_context(tc.tile_pool(name="const", bufs=1))
        psum = ctx.enter_context(tc.tile_pool(name="psum", bufs=4,
                                              space="PSUM"))
        psum_row = ctx.enter_context(tc.tile_pool(name="psum_row", bufs=2,
                                                  space="PSUM"))
        stage_cm = tc.tile_pool(name="stage", bufs=2)
        stage = stage_cm.__enter__()

        ones_row_bf = const.tile([1, 128], BF16)
        nc.vector.memset(ones_row_bf[:], 1.0)
        ones_row_f = const.tile([1, 128], F32)
        nc.vector.memset(ones_row_f[:], 1.0)
        ones_col_f = const.tile([128, 1], F32)
        nc.vector.memset(ones_col_f[:], 1.0)
        lneps_col = const.tile([128, 1], F32)
        nc.vector.memset(lneps_col[:], LN_EPS)

        W0_bf = const.tile([128, KC, D], BF16, tag="W0_bf")
        W1_bf = const.tile([128, KC, D], BF16, tag="W1_bf")
        for w_dram, w_bf in ((W0, W0_bf), (W1, W1_bf)):
            for k in range(KC):
                st = stage.tile([128, D], F32, tag="wstage")
                nc.sync.dma_start(st[:], w_dram[k * 128:(k + 1) * 128, :])
                nc.vector.tensor_copy(w_bf[:, k, :], st[:])

        We_bf = const.tile([128, SC, S], BF16, tag="We_bf")
        for c in range(SC):
            st = stage.tile([128, S], F32, tag="wstage")
            nc.sync.dma_start(st[:], We[c * 128:(c + 1) * 128, :])
            nc.vector.tensor_copy(We_bf[:, c, :], st[:])

        b0_row = const.tile([1, D], BF16, tag="b0_row")
        b1_row = const.tile([1, D], BF16, tag="b1_row")
        for b_dram, b_row in ((b0, b0_row), (b1, b1_row)):
            st = stage.tile([1, D], F32, tag="wstage")
            nc.sync.dma_start(st[:], b_dram[:])
            nc.vector.tensor_copy(b_row[:], st[:])

        wa_row_f = const.tile([1, D], F32, tag="wa_row_f")
        nc.sync.dma_start(wa_row_f[:], Wa[:, 0])
        wa_row_bf = const.tile([1, D], BF16, tag="wa_row_bf")
        nc.vector.tensor_copy(wa_row_bf[:], wa_row_f[:])
        Wa_bc = const.tile([128, D], BF16, tag="Wa_bc")
        for jt in range(JT):
            ps = psum.tile([128, 512], F32, tag="mm_ps")
            nc.tensor.matmul(ps[:], ones_row_bf[:],
                             wa_row_bf[0:1, jt * 512:(jt + 1) * 512])
            nc.vector.tensor_copy(Wa_bc[:, jt * 512:(jt + 1) * 512], ps[:])

        wa_col = const.tile([128, KC], F32, tag="wa_col")
        nc.sync.dma_start(wa_col[:], Wa[:, 0].rearrange("(k p) -> p k", p=128))
        wa_psum = stage.tile([128, 1], F32, tag="wa_psum")
        nc.vector.reduce_sum(wa_psum[:], wa_col[:], axis=mybir.AxisListType.X)
        ps = psum_row.tile([1, 512], F32, tag="row_ps")
        nc.tensor.matmul(ps[0:1, 0:1], ones_col_f[:], wa_psum[:])
        wab_row = stage.tile([1, 1], F32, tag="wab_row")
        nc.vector.tensor_copy(wab_row[:], ps[0:1, 0:1])
        ps2 = psum.tile([128, 512], F32, tag="mm_ps")
        nc.tensor.matmul(ps2[:, 0:1], ones_row_f[:], wab_row[:])
        negwab_col = const.tile([128, 1], F32, tag="negwab_col")
        nc.vector.tensor_scalar_mul(negwab_col[:], ps2[:, 0:1], -1.0)

        ba_row = stage.tile([1, 1], F32, tag="ba_row")
        nc.sync.dma_start(ba_row[:], ba[:])
        ps3 = psum.tile([128, 512], F32, tag="mm_ps")
        nc.tensor.matmul(ps3[:, 0:1], ones_row_f[:], ba_row[:])
        ba_col = const.tile([128, 1], F32, tag="ba_col")
        nc.vector.tensor_copy(ba_col[:], ps3[:, 0:1])

        be_row = const.tile([1, S], F32, tag="be_row")
        nc.sync.dma_start(be_row[:], be[:])
        stage_cm.__exit__(None, None, None)

        xtok_pool = ctx.enter_context(tc.tile_pool(name="xtok", bufs=2 * TC))
        xT_pool = ctx.enter_context(tc.tile_pool(name="xT", bufs=2))
        g_pool = ctx.enter_context(tc.tile_pool(name="g", bufs=4))
        lnT_pool = ctx.enter_context(tc.tile_pool(name="lnT", bufs=2))
        sq_pool = ctx.enter_context(tc.tile_pool(name="sq", bufs=4))
        st_pool = ctx.enter_context(tc.tile_pool(name="stats", bufs=12))
        row_pool = ctx.enter_context(tc.tile_pool(name="rows", bufs=2))
        cand_pool = ctx.enter_context(tc.tile_pool(name="cand", bufs=2))

        def dense_gelu_stats(actT, w_bf, b_row, tag):
            gs, ms, invs = [], [], []
            for t in range(TC):
                g = g_pool.tile([128, D], BF16, tag=f"g_{tag}")
                s1p = st_pool.tile([128, 2], F32, tag="s1p")
                s2p = st_pool.tile([128, 2], F32, tag="s2p")
                for jt in range(JT):
                    ps = psum.tile([128, 512], F32, tag="mm_ps")
                    for k in range(KC):
                        nc.tensor.matmul(
                            ps[:],
                            actT[:, k, t * 128:(t + 1) * 128],
                            w_bf[:, k, jt * 512:(jt + 1) * 512],
                            start=(k == 0), stop=False)
                    nc.tensor.matmul(ps[:], ones_row_bf[:],
                                     b_row[0:1, jt * 512:(jt + 1) * 512],
                                     start=False, stop=True)
                    nc.scalar.activation(g[:, jt * 512:(jt + 1) * 512], ps[:],
                                         AF.Gelu_apprx_tanh,
                                         accum_out=s1p[:, jt:jt + 1])
                    sq = sq_pool.tile([128, 512], BF16, tag=f"sq_{tag}")
                    nc.vector.scalar_tensor_tensor(
                        sq[:], g[:, jt * 512:(jt + 1) * 512], 1.0,
                        g[:, jt * 512:(jt + 1) * 512],
                        ALU.bypass, ALU.mult, accum_out=s2p[:, jt:jt + 1])
                s1 = st_pool.tile([128, 1], F32, tag="s1")
                nc.vector.tensor_tensor(s1[:], s1p[:, 0:1], s1p[:, 1:2],
                                        ALU.add)
                s2 = st_pool.tile([128, 1], F32, tag="s2")
                nc.vector.tensor_tensor(s2[:], s2p[:, 0:1], s2p[:, 1:2],
                                        ALU.add)
                m = st_pool.tile([128, 1], F32, tag="m")
                nc.vector.tensor_scalar_mul(m[:], s1[:], 1.0 / D)
                var = st_pool.tile([128, 1], F32, tag="var")
                negm = st_pool.tile([128, 1], F32, tag="negm")
                nc.vector.tensor_scalar_mul(negm[:], m[:], -1.0)
                q = st_pool.tile([128, 1], F32, tag="q")
                nc.vector.tensor_scalar_mul(q[:], s2[:], 1.0 / D)
                nc.vector.scalar_tensor_tensor(var[:], m[:], negm[:], q[:],
                                               ALU.mult, ALU.add)
                sd = st_pool.tile([128, 1], F32, tag="sd")
                nc.scalar.activation(sd[:], var[:], AF.Sqrt,
                                     bias=lneps_col[:])
                inv = st_pool.tile([128, 1], F32, tag="inv")
                nc.vector.reciprocal(inv[:], sd[:])
                gs.append(g)
                ms.append(m)
                invs.append(inv)
            return gs, ms, invs

        for e in range(bc):
            xtoks = []
            xT = xT_pool.tile([128, KC, S], BF16, tag="xT")
            for t in range(TC):
                xt = xtok_pool.tile([128, D], F32, tag="xtok")
                nc.sync.dma_start(xt[:], x[e, t * 128:(t + 1) * 128, :])
                xc = sq_pool.tile([128, D], BF16, tag="xcast")
                nc.vector.tensor_copy(xc[:], xt[:])
                for k in range(KC):
                    nc.sync.dma_start(
                        xT[:, k, t * 128:(t + 1) * 128],
                        xc[:, k * 128:(k + 1) * 128], transpose=True)
                xtoks.append(xt)

            g0s, m0s, inv0s = dense_gelu_stats(xT, W0_bf, b0_row, "l1")
            ln0T = lnT_pool.tile([128, KC, S], BF16, tag="ln0T")
            for t in range(TC):
                ln0 = g_pool.tile([128, D], BF16, tag="ln0")
                for jt in range(JT):
                    nc.vector.tensor_scalar(
                        ln0[:, jt * 512:(jt + 1) * 512],
                        g0s[t][:, jt * 512:(jt + 1) * 512],
                        m0s[t][:], inv0s[t][:], ALU.subtract, ALU.mult)
                for k in range(KC):
                    nc.sync.dma_start(
                        ln0T[:, k, t * 128:(t + 1) * 128],
                        ln0[:, k * 128:(k + 1) * 128], transpose=True)

            g1s, m1s, inv1s = dense_gelu_stats(ln0T, W1_bf, b1_row, "l2")

            alpha_cols = row_pool.tile([128, TC], F32, tag="alpha_cols")
            for t in range(TC):
                rp = st_pool.tile([128, 2], F32, tag="rp")
                for jt in range(JT):
                    sq = sq_pool.tile([128, 512], BF16, tag="adot")
                    nc.vector.scalar_tensor_tensor(
                        sq[:], g1s[t][:, jt * 512:(jt + 1) * 512], 1.0,
                        Wa_bc[:, jt * 512:(jt + 1) * 512],
                        ALU.bypass, ALU.mult, accum_out=rp[:, jt:jt + 1])
                r = st_pool.tile([128, 1], F32, tag="r")
                nc.vector.tensor_tensor(r[:], rp[:, 0:1], rp[:, 1:2], ALU.add)
                tmp = st_pool.tile([128, 1], F32, tag="tmp")
                nc.vector.scalar_tensor_tensor(tmp[:], m1s[t][:],
                                               negwab_col[:], r[:],
                                               ALU.mult, ALU.add)
                nc.vector.tensor_scalar(alpha_cols[:, t:t + 1], tmp[:],
                                        inv1s[t][:], ba_col[:],
                                        ALU.mult, ALU.add)

            alpha_row = row_pool.tile([1, S], F32, tag="alpha_row")
            for c in range(TC):
                nc.sync.dma_start(alpha_row[0:1, c * 128:(c + 1) * 128],
                                  alpha_cols[:, c:c + 1])
            mx = row_pool.tile([1, 1], F32, tag="mx")
            nc.vector.reduce_max(mx[:], alpha_row[:],
                                 axis=mybir.AxisListType.X)
            negmx = row_pool.tile([1, 1], F32, tag="negmx")
            nc.vector.tensor_scalar_mul(negmx[:], mx[:], -1.0)
            expv = row_pool.tile([1, S], F32, tag="expv")
            nc.scalar.activation(expv[:], alpha_row[:], AF.Exp,
                                 bias=negmx[:])
            ssum = row_pool.tile([1, 1], F32, tag="ssum")
            nc.vector.reduce_sum(ssum[:], expv[:], axis=mybir.AxisListType.X)
            rec = row_pool.tile([1, 1], F32, tag="rec")
            nc.vector.reciprocal(rec[:], ssum[:])
            alpha_sm = row_pool.tile([1, S], F32, tag="alpha_sm")
            nc.vector.tensor_scalar_mul(alpha_sm[:], expv[:], rec[:])

            asm_bf = row_pool.tile([1, S], BF16, tag="asm_bf")
            nc.vector.tensor_copy(asm_bf[:], alpha_sm[:])
            asm_cols = row_pool.tile([128, SC], BF16, tag="asm_cols")
            for c in range(SC):
                nc.sync.dma_start(asm_cols[:, c:c + 1],
                                  asm_bf[0:1, c * 128:(c + 1) * 128])
            eps_ps = psum_row.tile([1, S], F32, tag="row_ps")
            for c in range(SC):
                nc.tensor.matmul(eps_ps[:], asm_cols[:, c:c + 1],
                                 We_bf[:, c, :],
                                 start=(c == 0), stop=(c == SC - 1))
            eps_pre = row_pool.tile([1, S], F32, tag="eps_pre")
            nc.vector.tensor_tensor(eps_pre[:], eps_ps[:], be_row[:], ALU.add)
            eps_row = row_pool.tile([1, S], F32, tag="eps_row")
            nc.scalar.activation(eps_row[:], eps_pre[:], AF.Sigmoid)

            mask_row = row_pool.tile([1, S], F32, tag="mask_row")
            nc.vector.tensor_tensor(mask_row[:], alpha_sm[:], eps_row[:],
                                    ALU.is_gt)
            nc.sync.dma_start(g_out[e:e + 1, :], mask_row[:])
            mask_cols = row_pool.tile([128, TC], F32, tag="mask_cols")
            for c in range(TC):
                nc.sync.dma_start(mask_cols[:, c:c + 1],
                                  mask_row[0:1, c * 128:(c + 1) * 128])
            for t in range(TC):
                cand = cand_pool.tile([128, D], F32, tag="cand")
                nc.vector.tensor_scalar_mul(cand[:], xtoks[t][:],
                                            mask_cols[:, t:t + 1])
                nc.sync.dma_start(c_out[e, t * 128:(t + 1) * 128, :], cand[:])

    nc.compile()
    return nc


_CACHE = {}


def _get_nc():
    if "nc" not in _CACHE:
        _CACHE["nc"] = build_nc()
    return _CACHE["nc"]


def kernel(x, W0, b0, W1, b1, Wa, ba, We, be):
    nc = _get_nc()
    x = np.ascontiguousarray(np.asarray(x, dtype=np.float32))
    reps = dict(
        W0=np.asarray(W0, np.float32), b0=np.asarray(b0, np.float32),
        W1=np.asarray(W1, np.float32), b1=np.asarray(b1, np.float32),
        Wa=np.asarray(Wa, np.float32), ba=np.asarray(ba, np.float32),
        We=np.asarray(We, np.float32), be=np.asarray(be, np.float32),
    )
    in_maps = [dict(x=x[c * BC:(c + 1) * BC], **reps) for c in range(N_CORES)]
    res = run_bass_kernel_spmd(nc, in_maps, list(range(N_CORES)))
    greaters = np.concatenate([res.results[c]["greaters"]
                               for c in range(N_CORES)], axis=0) > 0.5
    candidate = np.concatenate([res.results[c]["candidate"]
                                for c in range(N_CORES)], axis=0)
    return greaters, candidate.astype(np.float32)


# revision 29
# speedup vs baseline: 1.6725x; 1.6725x over previous
import sys

if "/opt/trn_rl_repo" not in sys.path:
    sys.path.insert(0, "/opt/trn_rl_repo")

from contextlib import ExitStack

import numpy as np

import concourse.bacc as bacc
import concourse.tile as tile
from concourse import mybir
from concourse.bass_utils import run_bass_kernel_spmd

F32 = mybir.dt.float32
BF16 = mybir.dt.bfloat16
F8 = mybir.dt.float8e4
I32 = mybir.dt.int32
AF = mybir.ActivationFunctionType
ALU = mybir.AluOpType
DR = mybir.MatmulPerfMode.DoubleRow

USE_FP8 = True
MM_DT = F8 if USE_FP8 else BF16

N_CORES = 8
B, S, D = 64, 512, 1024
BC = B // N_CORES
KC = D // 128
TC = S // 128
JT = D // 512
SC = S // 128
LN_EPS = 1e-6
RSQRT_MAGIC = 0x5F3759DF


def build_nc(bc: int = BC, use_b0=True, use_b1=True, use_ba=True,
             use_be=True):
    nc = bacc.Bacc("TRN2", target_bir_lowering=False, debug=False,
                   num_devices=N_CORES)
    x = nc.dram_tensor("x", [bc, S, D], F32, kind="ExternalInput").ap()
    W0 = nc.dram_tensor("W0", [D, D], F32, kind="ExternalInput").ap()
    b0 = nc.dram_tensor("b0", [D], F32, kind="ExternalInput").ap()
    W1 = nc.dram_tensor("W1", [D, D], F32, kind="ExternalInput").ap()
    b1 = nc.dram_tensor("b1", [D], F32, kind="ExternalInput").ap()
    Wa = nc.dram_tensor("Wa", [D, 1], F32, kind="ExternalInput").ap()
    ba = nc.dram_tensor("ba", [1], F32, kind="ExternalInput").ap()
    We = nc.dram_tensor("We", [S, S], F32, kind="ExternalInput").ap()
    be = nc.dram_tensor("be", [S], F32, kind="ExternalInput").ap()
    g_out = nc.dram_tensor("greaters", [bc, S], F32,
                           kind="ExternalOutput").ap()
    c_out = nc.dram_tensor("candidate", [bc, S, D], F32,
                           kind="ExternalOutput").ap()

    with tile.TileContext(nc) as tc, ExitStack() as ctx:
        const = ctx.enter_context(tc.tile_pool(name="const", bufs=1))
        psum = ctx.enter_context(tc.tile_pool(name="psum", bufs=6,
                                              space="PSUM"))
        psum_row = ctx.enter_context(tc.tile_pool(name="psum_row", bufs=2,
                                                  space="PSUM"))
        xTb_pool = ctx.enter_context(tc.tile_pool(name="xTb", bufs=8))
        xT_pool = ctx.enter_context(tc.tile_pool(name="xT", bufs=2 * TC))
        g_pool = ctx.enter_context(tc.tile_pool(name="g", bufs=4))
        lnTb_pool = ctx.enter_context(tc.tile_pool(name="lnTb", bufs=8))
        lnT_pool = ctx.enter_context(tc.tile_pool(name="lnT", bufs=2 * TC))
        sq_pool = ctx.enter_context(tc.tile_pool(name="sq", bufs=4))
        st_pool = ctx.enter_context(tc.tile_pool(name="stats", bufs=3 * TC))
        row_pool = ctx.enter_context(tc.tile_pool(name="rows", bufs=2))
        cand_pool = ctx.enter_context(tc.tile_pool(name="cand", bufs=2))
        xc_pool = ctx.enter_context(tc.tile_pool(name="xc", bufs=4 * TC))

        def load_xT(e):
            xTs, xcs, xTbs = [], [], []
            for t in range(TC):
                xc = xc_pool.tile([128, D], BF16, tag="xcast")
                nc.gpsimd.dma_start(xc[:], x[e, t * 128:(t + 1) * 128, :])
                xcs.append(xc)
            for t in range(TC):
                xTb = xTb_pool.tile([128, KC, 128], BF16, tag="xTb")
                nc.sync.dma_start_transpose(xTb[:], xcs[t][:])
                xTbs.append(xTb)
            if USE_FP8:
                for t in range(TC):
                    xTf = xT_pool.tile([128, KC, 128], F8, tag="xTf")
                    nc.vector.tensor_copy(xTf[:], xTbs[t][:])
                    xTs.append(xTf)
            else:
                xTs = xTbs
            return xTs, xcs

        xT0 = load_xT(0)

        stage_cm = tc.tile_pool(name="stage", bufs=2)
        stage = stage_cm.__enter__()
        ones_row_bf = const.tile([1, 128], BF16)
        nc.vector.memset(ones_row_bf[:], 1.0)
        ones_row_f = const.tile([1, 128], F32)
        nc.vector.memset(ones_row_f[:], 1.0)
        ones_col_f = const.tile([128, 1], F32)
        nc.vector.memset(ones_col_f[:], 1.0)

        def load_w(w_dram, tag):
            tiles = []
            step = 2 if USE_FP8 else 1
            for c in range(KC // step):
                wt = const.tile([128, step, D], MM_DT, tag=f"{tag}_{c}")
                for s_ in range(step):
                    k = c * step + s_
                    nc.gpsimd.dma_start(wt[:, s_, :],
                                        w_dram[k * 128:(k + 1) * 128, :])
                tiles.append(wt)
            return tiles

        b0_row = const.tile([1, D], BF16, tag="b0_row")
        b1_row = const.tile([1, D], BF16, tag="b1_row")
        if use_b0:
            nc.gpsimd.dma_start(b0_row[:], b0[:])
        W0_t = load_w(W0, "W0")
        xT1 = load_xT(1) if bc > 1 else None

        We_t = []
        Wa_bc = []
        W1_t = []
        negwab_col = const.tile([128, 1], F32, tag="negwab_col")
        ba_col = None
        be_row = None
        if use_ba:
            ba_col = const.tile([128, 1], F32, tag="ba_col")
        if use_be:
            be_row = const.tile([1, S], F32, tag="be_row")

        def late_setup():
            if use_b1:
                nc.gpsimd.dma_start(b1_row[:], b1[:])
            W1_t.extend(load_w(W1, "W1"))
            for c in range(SC):
                wt = const.tile([128, S], BF16, tag=f"We_{c}")
                nc.gpsimd.dma_start(wt[:], We[c * 128:(c + 1) * 128, :])
                We_t.append(wt)

            wa_row_bf = const.tile([1, D], BF16, tag="wa_row_bf")
            nc.gpsimd.dma_start(wa_row_bf[:], Wa[:, 0])
            for jt in range(JT):
                wt = const.tile([128, 512], BF16, tag=f"Wabc_{jt}")
                ps = psum.tile([128, 512], F32, tag="mm_ps")
                nc.tensor.matmul(ps[:], ones_row_bf[:],
                                 wa_row_bf[0:1, jt * 512:(jt + 1) * 512])
                nc.vector.tensor_copy(wt[:], ps[:])
                Wa_bc.append(wt)

            wa_col = const.tile([128, KC], F32, tag="wa_col")
            nc.gpsimd.dma_start(wa_col[:],
                              Wa[:, 0].rearrange("(k p) -> p k", p=128))
            wa_psum = stage.tile([128, 1], F32, tag="wa_psum")
            nc.vector.reduce_sum(wa_psum[:], wa_col[:],
                                 axis=mybir.AxisListType.X)
            ps = psum_row.tile([1, 512], F32, tag="row_ps")
            nc.tensor.matmul(ps[0:1, 0:1], ones_col_f[:], wa_psum[:])
            wab_row = stage.tile([1, 1], F32, tag="wab_row")
            nc.vector.tensor_copy(wab_row[:], ps[0:1, 0:1])
            ps2 = psum.tile([128, 512], F32, tag="mm_ps")
            nc.tensor.matmul(ps2[:, 0:1], ones_row_f[:], wab_row[:])
            nc.vector.tensor_scalar_mul(negwab_col[:], ps2[:, 0:1], -1.0)

            if use_ba:
                ba_row = stage.tile([1, 1], F32, tag="ba_row")
                nc.gpsimd.dma_start(ba_row[:], ba[:])
                ps3 = psum.tile([128, 512], F32, tag="mm_ps")
                nc.tensor.matmul(ps3[:, 0:1], ones_row_f[:], ba_row[:])
                nc.vector.tensor_copy(ba_col[:], ps3[:, 0:1])

            if use_be:
                nc.gpsimd.dma_start(be_row[:], be[:])
            stage_cm.__exit__(None, None, None)

        def rsqrt_col(vcol, tag):
            vi = vcol.bitcast(I32)
            sh = st_pool.tile([128, 1], I32, tag=f"rs_sh_{tag}")
            nc.vector.tensor_scalar(sh[:], vi, 1, None, ALU.arith_shift_right)
            y0 = st_pool.tile([128, 1], I32, tag=f"rs_y0_{tag}")
            nc.vector.tensor_scalar(y0[:], sh[:], -1, RSQRT_MAGIC,
                                    ALU.mult, ALU.add)
            y = y0[:].bitcast(F32)
            for it in range(2):
                t1 = st_pool.tile([128, 1], F32, tag=f"rs_t1_{tag}_{it}")
                nc.vector.tensor_tensor(t1[:], y, y, ALU.mult)
                nc.vector.tensor_tensor(t1[:], t1[:], vcol, ALU.mult)
                nc.vector.tensor_scalar(t1[:], t1[:], -0.5, 1.5,
                                        ALU.mult, ALU.add)
                yn = st_pool.tile([128, 1], F32, tag=f"rs_yn_{tag}_{it}")
                nc.vector.tensor_tensor(yn[:], y, t1[:], ALU.mult)
                y = yn[:]
            return y

        def dense_gelu_stats(actT, w_t, b_row, use_b, tag, chunk_cb=None):
            gs, ms, invs = [], [], []
            for t in range(TC):
                gjt = []
                s1p = st_pool.tile([128, 2], F32, tag=f"s1p_{tag}")
                s2p = st_pool.tile([128, 2], F32, tag=f"s2p_{tag}")
                for jt in range(JT):
                    g = g_pool.tile([128, 512], BF16, tag=f"g_{tag}{jt}")
                    ps = psum.tile([128, 512], F32, tag="mm_ps")
                    if USE_FP8:
                        for c in range(KC // 2):
                            nc.tensor.matmul(
                                ps[:],
                                actT[t][:, 2 * c:2 * c + 2, :],
                                w_t[c][:, :, jt * 512:(jt + 1) * 512],
                                start=(c == 0),
                                stop=(c == KC // 2 - 1 and not use_b),
                                perf_mode=DR)
                    else:
                        for k in range(KC):
                            nc.tensor.matmul(
                                ps[:],
                                actT[t][:, k, :],
                                w_t[k][:, 0, jt * 512:(jt + 1) * 512],
                                start=(k == 0),
                                stop=(k == KC - 1 and not use_b))
                    if use_b:
                        nc.tensor.matmul(ps[:], ones_row_bf[:],
                                         b_row[0:1, jt * 512:(jt + 1) * 512],
                                         start=False, stop=True)
                    nc.scalar.activation(g[:], ps[:], AF.Gelu_apprx_tanh,
                                         accum_out=s1p[:, jt:jt + 1])
                    sq = sq_pool.tile([128, 512], BF16, tag=f"sq_{tag}")
                    nc.vector.scalar_tensor_tensor(
                        sq[:], g[:], 1.0, g[:], ALU.bypass, ALU.mult,
                        accum_out=s2p[:, jt:jt + 1])
                    gjt.append(g)
                s1 = st_pool.tile([128, 1], F32, tag=f"s1_{tag}")
                nc.vector.tensor_tensor(s1[:], s1p[:, 0:1], s1p[:, 1:2],
                                        ALU.add)
                s2 = st_pool.tile([128, 1], F32, tag=f"s2_{tag}")
                nc.vector.tensor_tensor(s2[:], s2p[:, 0:1], s2p[:, 1:2],
                                        ALU.add)
                m = st_pool.tile([128, 1], F32, tag=f"m_{tag}")
                nc.vector.tensor_scalar_mul(m[:], s1[:], 1.0 / D)
                negmsq = st_pool.tile([128, 1], F32, tag=f"negmsq_{tag}")
                nc.vector.scalar_tensor_tensor(negmsq[:], m[:], -1.0, m[:],
                                               ALU.mult, ALU.mult)
                var = st_pool.tile([128, 1], F32, tag=f"var_{tag}")
                nc.vector.tensor_scalar(var[:], s2[:], 1.0 / D, LN_EPS,
                                        ALU.mult, ALU.add)
                nc.vector.tensor_tensor(var[:], var[:], negmsq[:], ALU.add)
                inv = rsqrt_col(var[:], tag)
                gs.append(gjt)
                ms.append(m)
                invs.append(inv)
                if chunk_cb is not None:
                    chunk_cb(t, gjt, m[:], inv)
            return gs, ms, invs

        def stage_A(e, xTs):
            g0s, m0, inv0 = dense_gelu_stats(xTs, W0_t, b0_row, use_b0, "l1")
            ln0s, ln0Tbs, ln0Ts = [], [], []
            for t in range(TC):
                negminv = st_pool.tile([128, 1], F32, tag="negminv")
                nc.vector.tensor_scalar(negminv[:], m0[t][:], inv0[t], -1.0,
                                        ALU.mult, ALU.mult)
                perj = []
                for jt in range(JT):
                    ln0 = g_pool.tile([128, 512], BF16, tag=f"ln0_{jt}")
                    nc.scalar.activation(ln0[:], g0s[t][jt][:], AF.Identity,
                                         bias=negminv[:], scale=inv0[t])
                    perj.append(ln0)
                ln0s.append(perj)
            for t in range(TC):
                ln0Tb = lnTb_pool.tile([128, KC, 128], BF16, tag="ln0Tb")
                for jt in range(JT):
                    nc.sync.dma_start_transpose(
                        ln0Tb[:, jt * (KC // JT):(jt + 1) * (KC // JT), :],
                        ln0s[t][jt][:])
                ln0Tbs.append(ln0Tb)
            if USE_FP8:
                for t in range(TC):
                    ln0Tf = lnT_pool.tile([128, KC, 128], F8, tag="ln0Tf")
                    nc.vector.tensor_copy(ln0Tf[:], ln0Tbs[t][:])
                    ln0Ts.append(ln0Tf)
            else:
                ln0Ts = ln0Tbs
            return ln0Ts

        def stage_B(ln0Ts):
            alpha_cols = row_pool.tile([128, TC], F32, tag="alpha_cols")

            def alpha_cb(t, gjt, m_ap, inv_ap):
                rp = st_pool.tile([128, 2], F32, tag="rp")
                for jt in range(JT):
                    adot = sq_pool.tile([128, 512], BF16, tag="adot")
                    nc.vector.scalar_tensor_tensor(
                        adot[:], gjt[jt][:], 1.0, Wa_bc[jt][:],
                        ALU.bypass, ALU.mult, accum_out=rp[:, jt:jt + 1])
                r = st_pool.tile([128, 1], F32, tag="r")
                nc.vector.tensor_tensor(r[:], rp[:, 0:1], rp[:, 1:2], ALU.add)
                tmp = st_pool.tile([128, 1], F32, tag="tmp")
                nc.vector.scalar_tensor_tensor(tmp[:], m_ap, negwab_col[:],
                                               r[:], ALU.mult, ALU.add)
                if use_ba:
                    nc.vector.tensor_scalar(alpha_cols[:, t:t + 1], tmp[:],
                                            inv_ap, ba_col[:],
                                            ALU.mult, ALU.add)
                else:
                    nc.vector.tensor_scalar(alpha_cols[:, t:t + 1], tmp[:],
                                            inv_ap, None, ALU.mult)

            dense_gelu_stats(ln0Ts, W1_t, b1_row, use_b1, "l2",
                             chunk_cb=alpha_cb)
            return alpha_cols

        def stage_C_pre(e, alpha_cols):
            th = row_pool.tile([128, TC], F32, tag="th")
            nc.scalar.activation(th[:], alpha_cols[:], AF.Tanh, scale=0.5)
            den = row_pool.tile([128, TC], F32, tag="den")
            nc.vector.tensor_scalar(den[:], th[:], -1.0, 1.0,
                                    ALU.mult, ALU.add)
            nc.vector.tensor_scalar_max(den[:], den[:], 1e-30)
            nc.vector.reciprocal(den[:], den[:])
            nc.vector.tensor_scalar_add(th[:], th[:], 1.0)
            expv = row_pool.tile([128, TC], F32, tag="expv")
            nc.vector.tensor_tensor(expv[:], th[:], den[:], ALU.mult)
            expv_bf = row_pool.tile([128, TC], BF16, tag="expv_bf")
            nc.vector.tensor_copy(expv_bf[:], expv[:])

            expv_row = row_pool.tile([1, S], F32, tag="expv_row")
            for c in range(TC):
                nc.gpsimd.dma_start(expv_row[0:1, c * 128:(c + 1) * 128],
                                    expv[:, c:c + 1])
            ssum = row_pool.tile([1, 1], F32, tag="ssum")
            nc.vector.reduce_sum(ssum[:], expv_row[:],
                                 axis=mybir.AxisListType.X)
            rec = row_pool.tile([1, 1], F32, tag="rec")
            nc.vector.reciprocal(rec[:], ssum[:])
            alpha_sm = row_pool.tile([1, S], F32, tag="alpha_sm")
            nc.vector.tensor_scalar_mul(alpha_sm[:], expv_row[:], rec[:])
            return expv_bf, rec, alpha_sm

        def stage_C_mm(e, pre):
            expv_bf, rec, alpha_sm, xcs = pre
            eps_ps = psum_row.tile([1, S], F32, tag="row_ps")
            for c in range(SC):
                nc.tensor.matmul(eps_ps[:], expv_bf[:, c:c + 1], We_t[c][:],
                                 start=(c == 0), stop=(c == SC - 1))
            eps_row = row_pool.tile([1, S], F32, tag="eps_row")
            if use_be:
                nc.vector.scalar_tensor_tensor(eps_row[:], eps_ps[:], rec[:],
                                               be_row[:], ALU.mult, ALU.add)
            else:
                nc.vector.tensor_scalar_mul(eps_row[:], eps_ps[:], rec[:])
            nc.scalar.activation(eps_row[:], eps_row[:], AF.Tanh, scale=0.5)
            nc.vector.tensor_scalar(eps_row[:], eps_row[:], 0.5, 0.5,
                                    ALU.mult, ALU.add)

            mask_row = row_pool.tile([1, S], F32, tag="mask_row")
            nc.vector.tensor_tensor(mask_row[:], alpha_sm[:], eps_row[:],
                                    ALU.is_gt)
            nc.gpsimd.dma_start(g_out[e:e + 1, :], mask_row[:])
            for t in range(TC):
                mask_col = row_pool.tile([128, 1], F32, tag="mask_col")
                nc.gpsimd.dma_start(mask_col[:],
                                    mask_row[0:1, t * 128:(t + 1) * 128])
                cand = cand_pool.tile([128, D], F32, tag="cand")
                nc.vector.tensor_scalar_mul(cand[:], xcs[t][:], mask_col[:])
                nc.sync.dma_start(c_out[e, t * 128:(t + 1) * 128, :],
                                  cand[:])

        pend_xT = {0: xT0, 1: xT1}
        pend_B, pend_C, pend_Cm = {}, {}, {}
        for e in range(bc):
            if e + 2 < bc:
                pend_xT[e + 2] = load_xT(e + 2)
            if e - 2 in pend_C:
                ac_p, xcs_p2 = pend_C.pop(e - 2)
                pend_Cm[e - 2] = stage_C_pre(e - 2, ac_p) + (xcs_p2,)
            xTs_e, xcs_e = pend_xT.pop(e)
            pend_B[e] = (stage_A(e, xTs_e), xcs_e)
            if e == 0:
                late_setup()
            if e - 2 in pend_Cm:
                stage_C_mm(e - 2, pend_Cm.pop(e - 2))
            if e - 1 in pend_B:
                ln0Ts_p, xcs_p = pend_B.pop(e - 1)
                pend_C[e - 1] = (stage_B(ln0Ts_p), xcs_p)
        for e in (bc - 2, bc - 1):
            if e in pend_B:
                ln0Ts_p, xcs_p = pend_B.pop(e)
                pend_C[e] = (stage_B(ln0Ts_p), xcs_p)
            if e in pend_C:
                ac_p, xcs_p2 = pend_C.pop(e)
                pend_Cm[e] = stage_C_pre(e, ac_p) + (xcs_p2,)
            if e in pend_Cm:
                stage_C_mm(e, pend_Cm.pop(e))

    nc.compile()
    return nc


_CACHE = {}


def _get_nc(flags):
    if flags not in _CACHE:
        _CACHE[flags] = build_nc(BC, *flags)
    return _CACHE[flags]


def kernel(x, W0, b0, W1, b1, Wa, ba, We, be):
    x = np.ascontiguousarray(np.asarray(x, dtype=np.float32))
    reps = dict(
        W0=np.ascontiguousarray(np.asarray(W0, np.float32)),
        b0=np.ascontiguousarray(np.asarray(b0, np.float32)),
        W1=np.ascontiguousarray(np.asarray(W1, np.float32)),
        b1=np.ascontiguousarray(np.asarray(b1, np.float32)),
        Wa=np.ascontiguousarray(np.asarray(Wa, np.float32)),
        ba=np.ascontiguousarray(np.asarray(ba, np.float32)),
        We=np.ascontiguousarray(np.asarray(We, np.float32)),
        be=np.ascontiguousarray(np.asarray(be, np.float32)),
    )
    flags = (bool(reps["b0"].any()), bool(reps["b1"].any()),
             bool(reps["ba"].any()), bool(reps["be"].any()))
    nc = _get_nc(flags)
    in_maps = [dict(x=x[c * BC:(c + 1) * BC], **reps) for c in range(N_CORES)]
    res = run_bass_kernel_spmd(nc, in_maps, list(range(N_CORES)))
    greaters = np.concatenate([res.results[c]["greaters"]
                               for c in range(N_CORES)], axis=0) > 0.5
    candidate = np.concatenate([res.results[c]["candidate"]
                                for c in range(N_CORES)], axis=0)
    return greaters, candidate.astype(np.float32)
